# revision 20
# baseline (speedup 1.0000x reference)
"""EvolveGCN forward on 8 Trainium2 NeuronCores (Bass/Tile).

Strategy (graph/data parallel, per the sharding hint):
 - Nodes sharded contiguously across 8 cores (7500 real + 52 pad slots each).
 - Node features h kept FEATURE-MAJOR in SBUF: hT [128 feat x 7552 nodes].
 - Per (t, layer): hw = h @ W_eff computed as 59 stationary-load matmuls
   (lhsT = hT window slice) giving NODE-major hw tiles -> SBUF -> DRAM.
 - Send side: dma_gather of hw rows per out-edge in AllToAll-slot order
   (bucketed by destination owner), staged to the A2A input buffer.
 - AllToAll collective exchanges message rows between the 8 cores.
 - Receive side: dma_gather of received rows in dst-sorted 128-node-window-
   aligned order; per window one matmul aggT_w = msgs^T @ S_w (S carries the
   edge weights, built on host) performs scatter-sum + weighting + transpose
   back to feature-major in a single PE op.
 - relu on ACT engine PSUM->SBUF; end-of-timestep touched-mask select on DVE.
 - Evolved weights (LSTM chain over the 128x128 GCN weights) are a pure
   function of the parameters; computed on host, replicated to all cores.
 - Final graph max-pool + output head + BCE computed on host from the
   returned per-core hT (the gather/unshard step).
"""
import contextlib
import os

import ml_dtypes
import numpy as np

import concourse.bass as bass
import concourse.bacc as bacc
import concourse.mybir as mybir
import concourse.tile as tile
from concourse.bass_utils import run_bass_kernel_spmd

# problem constants (hardcoded per spec)
N, E, D, NINP, V, T, L, B = 60000, 200000, 128, 256, 15000, 6, 2, 16
NCORES = 8
NPC = N // NCORES          # 7500 real nodes per core
P = 128
NW = 59                    # windows per core (59*128 = 7552 slots)
NS = NW * P                # 7552 padded node slots per core
GCH = 1024                 # dma_gather chunk (indices per call)

f32 = mybir.dt.float32
u8 = mybir.dt.uint8
i32 = mybir.dt.int32
i16 = mybir.dt.int16
bf16 = mybir.dt.bfloat16

LAST_EXEC_NS = None
LAST_NC = None


# ---------------------------------------------------------------------------
# host-side math (exact replica of reference semantics, numpy fp32)
# ---------------------------------------------------------------------------

def _sigmoid(x):
    return 1.0 / (1.0 + np.exp(-x))


def _lstm_cell(x, h, c, Wih, Whh, bih, bhh):
    g = x @ Wih.T + bih + h @ Whh.T + bhh
    i, f, gg, o = np.split(g, 4, axis=-1)
    c2 = _sigmoid(f) * c + _sigmoid(i) * np.tanh(gg)
    return _sigmoid(o) * np.tanh(c2), c2


def _evolve_weights(edge_time, gcn_weights, lstm_Wih, lstm_Whh, lstm_bih, lstm_bhh):
    """W_eff[t][i]: the evolved GCN weight used at step t, layer i."""
    hx = [gcn_weights[i].copy() for i in range(L)]
    cx = [np.zeros((D, D), np.float32) for _ in range(L)]
    z = np.zeros((D, D), np.float32)
    first = False
    W_eff = []
    for t in range(T):
        has = bool(np.any(edge_time == t))
        row = []
        for i in range(L):
            h_zero, c_zero = _lstm_cell(hx[i], z, z, lstm_Wih[i], lstm_Whh[i],
                                        lstm_bih[i], lstm_bhh[i])
            h_st, _ = _lstm_cell(hx[i], hx[i], cx[i], lstm_Wih[i], lstm_Whh[i],
                                 lstm_bih[i], lstm_bhh[i])
            hi = h_st if first else h_zero
            ci = cx[i] if first else c_zero
            if not has:
                hi = hx[i]
                ci = cx[i]
            hx[i] = np.asarray(hi, np.float32)
            cx[i] = np.asarray(ci, np.float32)
            row.append(hx[i])
        W_eff.append(row)
        first = first or has
    return W_eff


def _wrap16(idx):
    """Pack an index list into the [128, n/16] wrapped SBUF layout used by
    dma_gather: index i lands at [i % 16, i // 16]; rows 16..127 zero."""
    n = idx.size
    assert n % 16 == 0
    return np.ascontiguousarray(
        np.tile(idx.reshape(n // 16, 16).T.astype(np.int16), (8, 1)))


# ---------------------------------------------------------------------------
# host-side graph preprocessing
# ---------------------------------------------------------------------------

def _preprocess_edges(src, dst, edge_time, edge_weight):
    """Static per-t structures; sizes uniform across cores."""
    meta = {"t_active": [], "BK": {}, "NT": {}, "tpw": {}}
    percore = [dict(sendidx={}, sendidx32={}, perm={}, perm32={}, smat={}, mask={}) for _ in range(NCORES)]

    for t in range(T):
        sel = np.where(edge_time == t)[0]
        if sel.size == 0:
            continue
        meta["t_active"].append(t)
        s = src[sel].astype(np.int64)
        d = dst[sel].astype(np.int64)
        w = edge_weight[sel].astype(np.float32)
        so = s // NPC
        do = d // NPC
        sl = s % NPC
        dl = d % NPC

        bucket_id = so * NCORES + do
        order = np.argsort(bucket_id, kind="stable")
        bid_sorted = bucket_id[order]
        cnt = np.bincount(bid_sorted, minlength=NCORES * NCORES)
        BK = int(((cnt.max() + 15) // 16) * 16)
        meta["BK"][t] = BK
        start = np.zeros(NCORES * NCORES + 1, np.int64)
        np.cumsum(cnt, out=start[1:])
        rank = np.empty(sel.size, np.int64)
        rank[order] = np.arange(sel.size) - start[bid_sorted]
        send_slot = do * BK + rank     # slot in sender's a2a_in
        recv_row = so * BK + rank      # row in receiver's a2a_out
        SBK = NCORES * BK              # multiple of 128

        # ---- send side: gather index list in slot order ----
        for c in range(NCORES):
            m = so == c
            sidx = np.zeros(SBK, np.int64)   # pad -> slot 0 (finite garbage)
            sidx[send_slot[m]] = sl[m]
            percore[c]["sendidx"][t] = _wrap16(sidx)
            percore[c]["sendidx32"][t] = np.ascontiguousarray(
                sidx.reshape(SBK // P, P).T.astype(np.int32))

        # ---- recv side: window-aligned dst-sorted tiles ----
        tiles_pw = np.zeros((NCORES, NW), np.int64)
        recv_data = []
        for c in range(NCORES):
            m = do == c
            o3 = np.argsort(dl[m], kind="stable")
            dl_s = dl[m][o3]
            row_s = recv_row[m][o3]
            w_s = w[m][o3]
            win = dl_s // P
            tiles_pw[c] = np.bincount(win, minlength=NW)
            recv_data.append((dl_s, row_s, w_s, win))
        tpw = np.maximum(1, -(-np.max(tiles_pw, axis=0) // P)).astype(np.int64)
        meta["tpw"][t] = tpw
        NT = int(tpw.sum())
        meta["NT"][t] = NT

        tile_base = np.zeros(NW + 1, np.int64)
        np.cumsum(tpw, out=tile_base[1:])
        for c in range(NCORES):
            dl_s, row_s, w_s, win = recv_data[c]
            perm = np.zeros(NT * P, np.int64)          # pad -> row 0
            smat = np.zeros((NT, P, P), np.float32)
            counts = np.zeros(NW, np.int64)
            ti = tile_base[win] + counts[win] // P     # placeholder; loop below
            counts = np.zeros(NW, np.int64)
            for k in range(dl_s.size):
                wi = win[k]
                pos = counts[wi]
                counts[wi] += 1
                tile_i = tile_base[wi] + pos // P
                mrow = pos % P
                perm[tile_i * P + mrow] = row_s[k]
                smat[tile_i, mrow, dl_s[k] % P] += w_s[k]
            percore[c]["perm"][t] = _wrap16(perm)
            percore[c]["perm32"][t] = np.ascontiguousarray(
                perm.reshape(NT, P).T.astype(np.int32))
            # smat -> [128, NT*128]: [p, j*128+col] = smat[j, p, col]
            percore[c]["smat"][t] = np.ascontiguousarray(
                np.transpose(smat, (1, 0, 2)).reshape(P, NT * P))

        # ---- touched mask ----
        for c in range(NCORES):
            deg = np.zeros(NS, np.float32)
            np.add.at(deg, sl[so == c], 1.0)
            np.add.at(deg, dl[do == c], 1.0)
            percore[c]["mask"][t] = (deg > 0).astype(np.uint8)

    return meta, percore


# ---------------------------------------------------------------------------
# device kernel builder
# ---------------------------------------------------------------------------

def _build(meta):
    STAGE = int(os.environ.get("KERNEL_STAGE", "5"))
    GMODE = os.environ.get("GMODE", "gather")
    nc = bacc.Bacc("TRN2", target_bir_lowering=False, debug=False,
                   enable_asserts=True, num_devices=NCORES)

    t_act = meta["t_active"]
    sumSB = sum(NCORES * meta["BK"][t] for t in t_act)      # send idx count
    sumNT = sum(meta["NT"][t] for t in t_act)
    BKmax = max(meta["BK"][t] for t in t_act) if t_act else 16
    NTmax = max(meta["NT"][t] for t in t_act) if t_act else 1
    SBKmax = NCORES * BKmax

    emb_in = nc.dram_tensor("embT", [2 * P, NS], f32, kind="ExternalInput").ap()
    aw_in = nc.dram_tensor("adaptW", [P, 2 * P], f32, kind="ExternalInput").ap()
    ab_in = nc.dram_tensor("adaptB", [P, 1], f32, kind="ExternalInput").ap()
    weff_in = nc.dram_tensor("weff", [P, T * L * P], f32, kind="ExternalInput").ap()
    sidx_in = nc.dram_tensor("sendidx", [P, max(1, sumSB // 16)], i16,
                             kind="ExternalInput").ap()
    sidx32_in = nc.dram_tensor("sendidx32", [P, max(1, sumSB // P)], i32,
                               kind="ExternalInput").ap()
    perm32_in = nc.dram_tensor("perm32", [P, max(1, sumNT)], i32,
                               kind="ExternalInput").ap()
    perm_in = nc.dram_tensor("perm", [P, max(1, sumNT * P // 16)], i16,
                             kind="ExternalInput").ap()
    smat_in = nc.dram_tensor("smat", [P, max(1, sumNT) * P], bf16,
                             kind="ExternalInput").ap()
    mask_in = nc.dram_tensor("mask", [P, max(1, len(t_act)) * NS], u8,
                             kind="ExternalInput").ap()

    hT_out = nc.dram_tensor("hT_out", [P, NS], f32, kind="ExternalOutput").ap()

    GW = max(NS, NTmax * P)

    with tile.TileContext(nc) as tc:
        with (
            tc.tile_pool(name="per", bufs=1) as per,
            tc.tile_pool(name="dram", bufs=1, space="DRAM") as dram,
        ):
            hT = per.tile([P, NS], f32)
            hS = per.tile([P, NS], f32)
            hwG = per.tile([P, GW], bf16)      # hw tiles, then gathered msgs
            msb = per.tile([P, SBKmax], bf16)  # staged send messages
            S = per.tile([P, NTmax * P], bf16)
            msk = per.tile([P, NS], u8)
            weff = per.tile([P, T * L * P], f32)
            adb = per.tile([P, 1], f32)

            nc.sync.dma_start(out=weff[:], in_=weff_in[:])
            nc.sync.dma_start(out=adb[:], in_=ab_in[:])

            hw_dram = dram.tile([NS, P], bf16)
            a2a_in = dram.tile([SBKmax, P], bf16)
            a2a_out = dram.tile([SBKmax, P], bf16)

            # ---------------- h0 = embT @ adapt_W + b (two passes) --------
            with (
                tc.tile_pool(name="h0sb", bufs=1) as h0sb,
                tc.tile_pool(name="h0ps", bufs=3, space="PSUM") as h0ps,
            ):
                awt = h0sb.tile([P, 2 * P], f32)
                nc.sync.dma_start(out=awt[:], in_=aw_in[:])
                embT = h0sb.tile([P, NS], f32)
                for ch in range(2):
                    nc.sync.dma_start(out=embT[:], in_=emb_in[ch * P:(ch + 1) * P, :])
                    for wq in range(15):
                        c0 = wq * 512
                        cw = min(512, NS - c0)
                        ps0 = h0ps.tile([P, 512], f32, space="PSUM", tag="ps0")
                        nc.tensor.matmul(out=ps0[:, :cw],
                                         lhsT=awt[:, ch * P:(ch + 1) * P],
                                         rhs=embT[:, c0:c0 + cw],
                                         start=True, stop=True)
                        if ch == 0:
                            nc.vector.tensor_scalar_add(
                                out=hT[:, c0:c0 + cw], in0=ps0[:, :cw],
                                scalar1=adb[:, :1])
                        else:
                            nc.vector.tensor_add(out=hT[:, c0:c0 + cw],
                                                 in0=hT[:, c0:c0 + cw],
                                                 in1=ps0[:, :cw])

            # ---------------- T x L loop ----------------
            with (
                tc.tile_pool(name="ld", bufs=2) as ld,
                tc.tile_pool(name="tmp", bufs=4) as tmp,
                tc.tile_pool(name="pshw", bufs=4, space="PSUM") as pshw,
                tc.tile_pool(name="psagg", bufs=4, space="PSUM") as psagg,
            ):
                sb_base = 0
                nt_base = 0
                for kt, t in enumerate(t_act):
                    BK = meta["BK"][t]
                    NT = meta["NT"][t]
                    tpw = meta["tpw"][t]
                    SBK = NCORES * BK

                    nc.sync.dma_start(
                        out=S[:, :NT * P],
                        in_=smat_in[:, nt_base * P:(nt_base + NT) * P])
                    nc.sync.dma_start(
                        out=msk[:], in_=mask_in[:, kt * NS:(kt + 1) * NS])
                    sidxt = ld.tile([P, SBK // 16], i16, tag="sidxt")
                    nc.sync.dma_start(
                        out=sidxt[:],
                        in_=sidx_in[:, sb_base // 16:(sb_base + SBK) // 16])
                    permt = ld.tile([P, NT * P // 16], i16, tag="permt")
                    nc.sync.dma_start(
                        out=permt[:],
                        in_=perm_in[:, nt_base * P // 16:(nt_base + NT) * P // 16])
                    sidx32t = ld.tile([P, SBK // P], i32, tag="sidx32t")
                    nc.sync.dma_start(
                        out=sidx32t[:],
                        in_=sidx32_in[:, sb_base // P:(sb_base + SBK) // P])
                    perm32t = ld.tile([P, NT], i32, tag="perm32t")
                    nc.sync.dma_start(
                        out=perm32t[:],
                        in_=perm32_in[:, nt_base:nt_base + NT])

                    for li in range(L):
                        hsrc = hT if (li == 0 or STAGE < 5) else hS
                        Wt = weff[:, (t * L + li) * P:(t * L + li + 1) * P]

                        # 1) hw = h @ W_eff (node-major tiles) -> SBUF -> DRAM
                        for wq in range(NW):
                            ps = pshw.tile([P, P], f32, space="PSUM", tag="pshw")
                            nc.tensor.matmul(out=ps[:],
                                             lhsT=hsrc[:, wq * P:(wq + 1) * P],
                                             rhs=Wt, start=True, stop=True)
                            nc.vector.tensor_copy(
                                out=hwG[:, wq * P:(wq + 1) * P], in_=ps[:])
                        nc.sync.dma_start(
                            out=hw_dram[:].rearrange("(w q) f -> q w f", q=P),
                            in_=hwG[:, :NS].rearrange("p (w f) -> p w f", f=P))

                        # 2) send-side gather into slot order + stage to a2a_in
                        if STAGE < 2:
                            continue
                        if GMODE == "indirect":
                            for b in range(SBK // P):
                                nc.gpsimd.indirect_dma_start(
                                    out=msb[:, b * P:(b + 1) * P],
                                    out_offset=None,
                                    in_=hw_dram[:],
                                    in_offset=bass.IndirectOffsetOnAxis(
                                        ap=sidx32t[:, b:b + 1], axis=0),
                                )
                        else:
                          for g0 in range(0, SBK, GCH):
                            gn = min(GCH, SBK - g0)
                            nc.gpsimd.dma_gather(
                                out_ap=msb[:, g0:g0 + gn].rearrange(
                                    "p (j f) -> p j f", f=P),
                                in_ap=hw_dram[:],
                                idxs_ap=sidxt[:, g0 // 16:(g0 + gn) // 16],
                                num_idxs=gn,
                                num_idxs_reg=gn,
                                elem_size=P,
                            )
                        del g0
                        if STAGE < 3:
                            continue
                        nc.sync.dma_start(
                            out=a2a_in[0:SBK, :].rearrange("(j q) f -> q j f", q=P),
                            in_=msb[:, :SBK].rearrange("p (j f) -> p j f", f=P))

                        # 3) AllToAll
                        nc.gpsimd.collective_compute(
                            "AllToAll", mybir.AluOpType.bypass,
                            replica_groups=[list(range(NCORES))],
                            ins=[a2a_in[0:SBK, :]],
                            outs=[a2a_out[0:SBK, :]],
                        )

                        # 4) recv-side gather (dst-sorted window-aligned)
                        if STAGE < 4:
                            continue
                        if GMODE == "indirect":
                            for j in range(NT):
                                nc.gpsimd.indirect_dma_start(
                                    out=hwG[:, j * P:(j + 1) * P],
                                    out_offset=None,
                                    in_=a2a_out[0:SBK, :],
                                    in_offset=bass.IndirectOffsetOnAxis(
                                        ap=perm32t[:, j:j + 1], axis=0),
                                )
                        else:
                          for g0 in range(0, NT * P, GCH):
                            gn = min(GCH, NT * P - g0)
                            nc.gpsimd.dma_gather(
                                out_ap=hwG[:, g0:g0 + gn].rearrange(
                                    "p (j f) -> p j f", f=P),
                                in_ap=a2a_out[0:SBK, :],
                                idxs_ap=permt[:, g0 // 16:(g0 + gn) // 16],
                                num_idxs=gn,
                                num_idxs_reg=gn,
                                elem_size=P,
                            )

                        # 5) S-matmuls + relu (+ mask merge on last layer)
                        if STAGE < 5:
                            continue
                        jj = 0
                        for wq in range(NW):
                            k = int(tpw[wq])
                            ps = psagg.tile([P, P], f32, space="PSUM", tag="psagg")
                            for u in range(k):
                                nc.tensor.matmul(
                                    out=ps[:],
                                    lhsT=hwG[:, (jj + u) * P:(jj + u + 1) * P],
                                    rhs=S[:, (jj + u) * P:(jj + u + 1) * P],
                                    start=(u == 0), stop=(u == k - 1))
                            jj += k
                            sl_ = slice(wq * P, (wq + 1) * P)
                            if li < L - 1:
                                nc.scalar.activation(
                                    hS[:, sl_], ps[:],
                                    mybir.ActivationFunctionType.Relu)
                            else:
                                rt = tmp.tile([P, P], f32, tag="rt")
                                nc.scalar.activation(
                                    rt[:], ps[:],
                                    mybir.ActivationFunctionType.Relu)
                                nc.vector.select(hT[:, sl_], msk[:, sl_],
                                                 rt[:], hT[:, sl_])

                    sb_base += SBK
                    nt_base += NT

                nc.sync.dma_start(out=hT_out[:], in_=hT[:])

    nc.finalize()
    return nc


# ---------------------------------------------------------------------------
# top-level kernel
# ---------------------------------------------------------------------------

def kernel(word_ids, src, dst, edge_time, edge_weight, graph_id, y_data,
           word_embeds, adapt_W, adapt_b, gcn_weights,
           lstm_Wih, lstm_Whh, lstm_bih, lstm_bhh, out_W, out_b):
    global LAST_EXEC_NS, LAST_NC

    word_ids = np.asarray(word_ids).astype(np.int64)
    src = np.asarray(src).astype(np.int64)
    dst = np.asarray(dst).astype(np.int64)
    edge_time = np.asarray(edge_time).astype(np.int64)
    edge_weight = np.asarray(edge_weight).astype(np.float32)
    graph_id = np.asarray(graph_id).astype(np.int64)
    y_data = np.asarray(y_data).astype(np.float32)
    word_embeds = np.asarray(word_embeds).astype(np.float32)
    adapt_W = np.asarray(adapt_W).astype(np.float32)
    adapt_b = np.asarray(adapt_b).astype(np.float32)
    gcn_weights = np.asarray(gcn_weights).astype(np.float32)
    lstm_Wih = np.asarray(lstm_Wih).astype(np.float32)
    lstm_Whh = np.asarray(lstm_Whh).astype(np.float32)
    lstm_bih = np.asarray(lstm_bih).astype(np.float32)
    lstm_bhh = np.asarray(lstm_bhh).astype(np.float32)
    out_W = np.asarray(out_W).astype(np.float32)
    out_b = np.asarray(out_b).astype(np.float32)

    W_eff = _evolve_weights(edge_time, gcn_weights, lstm_Wih, lstm_Whh,
                            lstm_bih, lstm_bhh)
    meta, percore = _preprocess_edges(src, dst, edge_time, edge_weight)
    trunc = os.environ.get("KERNEL_TRUNC")
    if trunc is not None:
        meta["t_active"] = meta["t_active"][:int(trunc)]
    t_act = meta["t_active"]

    weff_sw = np.zeros((P, T * L * P), np.float32)
    for t in range(T):
        for i in range(L):
            weff_sw[:, (t * L + i) * P:(t * L + i + 1) * P] = W_eff[t][i]
    aw_sw = np.ascontiguousarray(
        adapt_W.reshape(2, P, P).transpose(1, 0, 2).reshape(P, 2 * P))
    ab = np.ascontiguousarray(adapt_b.reshape(P, 1))

    sumSB = sum(NCORES * meta["BK"][t] for t in t_act)
    sumNT = sum(meta["NT"][t] for t in t_act)
    nmask = max(1, len(t_act))

    in_maps = []
    for c in range(NCORES):
        wid = word_ids[c * NPC:(c + 1) * NPC]
        embT = np.zeros((2 * P, NS), np.float32)
        ge = word_embeds[wid]
        embT[0:P, 0:NPC] = ge[:, 0:P].T
        embT[P:2 * P, 0:NPC] = ge[:, P:2 * P].T

        sidx = np.zeros((128, max(1, sumSB // 16)), np.int16)
        sidx32 = np.zeros((128, max(1, sumSB // P)), np.int32)
        perm32c = np.zeros((128, max(1, sumNT)), np.int32)
        permc = np.zeros((128, max(1, sumNT * P // 16)), np.int16)
        smatc = np.zeros((P, max(1, sumNT) * P), ml_dtypes.bfloat16)
        maskc = np.zeros((P, nmask * NS), np.uint8)
        sb = 0
        ntb = 0
        for kt, t in enumerate(t_act):
            SBK = NCORES * meta["BK"][t]
            NT = meta["NT"][t]
            sidx[:, sb // 16:(sb + SBK) // 16] = percore[c]["sendidx"][t]
            sidx32[:, sb // P:(sb + SBK) // P] = percore[c]["sendidx32"][t]
            perm32c[:, ntb:ntb + NT] = percore[c]["perm32"][t]
            permc[:, ntb * P // 16:(ntb + NT) * P // 16] = percore[c]["perm"][t]
            smatc[:, ntb * P:(ntb + NT) * P] = percore[c]["smat"][t]
            maskc[:, kt * NS:(kt + 1) * NS] = percore[c]["mask"][t][None, :]
            sb += SBK
            ntb += NT

        in_maps.append({
            "embT": embT, "adaptW": aw_sw, "adaptB": ab, "weff": weff_sw,
            "sendidx": sidx, "sendidx32": sidx32, "perm": permc,
            "perm32": perm32c, "smat": smatc, "mask": maskc,
        })

    nc = _build(meta)
    LAST_NC = nc
    prof_ctx = contextlib.nullcontext()
    prof_dir = os.environ.get("KERNEL_PROF_DIR")
    if prof_dir:
        try:
            from trn_agent_boot.trn_boot import _ntff_profile_via_ctypes
            hook = _ntff_profile_via_ctypes("/opt/axon/libaxon_pjrt.so")
            if hook is not None:
                os.makedirs(prof_dir, exist_ok=True)
                prof_ctx = hook(prof_dir, None)
        except Exception as e:  # profiling is best-effort only
            print(f"profiling hook unavailable: {e}")
    with prof_ctx:
        res = run_bass_kernel_spmd(nc, in_maps, core_ids=list(range(NCORES)))
    LAST_EXEC_NS = res.exec_time_ns

    h = np.zeros((N, D), np.float32)
    for c in range(NCORES):
        h[c * NPC:(c + 1) * NPC] = res.results[c]["hT_out"][:, :NPC].T

    pooled = np.full((B, D), -np.inf, np.float32)
    np.maximum.at(pooled, graph_id, h)
    pooled = np.where(np.isfinite(pooled), pooled, 0.0).astype(np.float32)
    logits = (pooled @ out_W + out_b).reshape(-1).astype(np.float32)
    loss = np.mean(np.maximum(logits, 0.0) - logits * y_data +
                   np.log1p(np.exp(-np.abs(logits)))).astype(np.float32)
    probs = _sigmoid(logits).astype(np.float32)
    return loss, probs


# revision 21
# speedup vs baseline: 1.0360x; 1.0360x over previous
"""EvolveGCN forward on 8 Trainium2 NeuronCores (Bass/Tile).

Strategy (graph/data parallel, per the sharding hint):
 - Nodes sharded contiguously across 8 cores (7500 real + 52 pad slots each).
 - Node features h kept FEATURE-MAJOR in SBUF: hT [128 feat x 7552 nodes].
 - Per (t, layer): hw = h @ W_eff computed as 59 stationary-load matmuls
   (lhsT = hT window slice) giving NODE-major hw tiles -> SBUF -> DRAM.
 - Send side: dma_gather of hw rows per out-edge in AllToAll-slot order
   (bucketed by destination owner), staged to the A2A input buffer.
 - AllToAll collective exchanges message rows between the 8 cores.
 - Receive side: dma_gather of received rows in dst-sorted 128-node-window-
   aligned order; per window one matmul aggT_w = msgs^T @ S_w (S carries the
   edge weights, built on host) performs scatter-sum + weighting + transpose
   back to feature-major in a single PE op.
 - relu on ACT engine PSUM->SBUF; end-of-timestep touched-mask select on DVE.
 - Evolved weights (LSTM chain over the 128x128 GCN weights) are a pure
   function of the parameters; computed on host, replicated to all cores.
 - Final graph max-pool + output head + BCE computed on host from the
   returned per-core hT (the gather/unshard step).
"""
import contextlib
import os

import ml_dtypes
import numpy as np

import concourse.bass as bass
import concourse.bacc as bacc
import concourse.mybir as mybir
import concourse.tile as tile
from concourse.bass_utils import run_bass_kernel_spmd

# problem constants (hardcoded per spec)
N, E, D, NINP, V, T, L, B = 60000, 200000, 128, 256, 15000, 6, 2, 16
NCORES = 8
NPC = N // NCORES          # 7500 real nodes per core
P = 128
NW = 59                    # windows per core (59*128 = 7552 slots)
NS = NW * P                # 7552 padded node slots per core
GCH = 1024                 # dma_gather chunk (indices per call)

f32 = mybir.dt.float32
u8 = mybir.dt.uint8
i32 = mybir.dt.int32
i16 = mybir.dt.int16
bf16 = mybir.dt.bfloat16

LAST_EXEC_NS = None
LAST_NC = None


# ---------------------------------------------------------------------------
# host-side math (exact replica of reference semantics, numpy fp32)
# ---------------------------------------------------------------------------

def _sigmoid(x):
    return 1.0 / (1.0 + np.exp(-x))


def _lstm_cell(x, h, c, Wih, Whh, bih, bhh):
    g = x @ Wih.T + bih + h @ Whh.T + bhh
    i, f, gg, o = np.split(g, 4, axis=-1)
    c2 = _sigmoid(f) * c + _sigmoid(i) * np.tanh(gg)
    return _sigmoid(o) * np.tanh(c2), c2


def _evolve_weights(edge_time, gcn_weights, lstm_Wih, lstm_Whh, lstm_bih, lstm_bhh):
    """W_eff[t][i]: the evolved GCN weight used at step t, layer i."""
    hx = [gcn_weights[i].copy() for i in range(L)]
    cx = [np.zeros((D, D), np.float32) for _ in range(L)]
    z = np.zeros((D, D), np.float32)
    first = False
    W_eff = []
    for t in range(T):
        has = bool(np.any(edge_time == t))
        row = []
        for i in range(L):
            h_zero, c_zero = _lstm_cell(hx[i], z, z, lstm_Wih[i], lstm_Whh[i],
                                        lstm_bih[i], lstm_bhh[i])
            h_st, _ = _lstm_cell(hx[i], hx[i], cx[i], lstm_Wih[i], lstm_Whh[i],
                                 lstm_bih[i], lstm_bhh[i])
            hi = h_st if first else h_zero
            ci = cx[i] if first else c_zero
            if not has:
                hi = hx[i]
                ci = cx[i]
            hx[i] = np.asarray(hi, np.float32)
            cx[i] = np.asarray(ci, np.float32)
            row.append(hx[i])
        W_eff.append(row)
        first = first or has
    return W_eff


def _wrap16(idx):
    """Pack an index list into the [128, n/16] wrapped SBUF layout used by
    dma_gather: index i lands at [i % 16, i // 16]; rows 16..127 zero."""
    n = idx.size
    assert n % 16 == 0
    return np.ascontiguousarray(
        np.tile(idx.reshape(n // 16, 16).T.astype(np.int16), (8, 1)))


# ---------------------------------------------------------------------------
# host-side graph preprocessing
# ---------------------------------------------------------------------------

def _preprocess_edges(src, dst, edge_time, edge_weight):
    """Static per-t structures; sizes uniform across cores."""
    meta = {"t_active": [], "BK": {}, "NT": {}, "tpw": {}}
    percore = [dict(sendidx={}, sendidx32={}, perm={}, perm32={}, smat={}, mask={}) for _ in range(NCORES)]

    for t in range(T):
        sel = np.where(edge_time == t)[0]
        if sel.size == 0:
            continue
        meta["t_active"].append(t)
        s = src[sel].astype(np.int64)
        d = dst[sel].astype(np.int64)
        w = edge_weight[sel].astype(np.float32)
        so = s // NPC
        do = d // NPC
        sl = s % NPC
        dl = d % NPC

        bucket_id = so * NCORES + do
        order = np.argsort(bucket_id, kind="stable")
        bid_sorted = bucket_id[order]
        cnt = np.bincount(bid_sorted, minlength=NCORES * NCORES)
        BK = int(((cnt.max() + 15) // 16) * 16)
        meta["BK"][t] = BK
        start = np.zeros(NCORES * NCORES + 1, np.int64)
        np.cumsum(cnt, out=start[1:])
        rank = np.empty(sel.size, np.int64)
        rank[order] = np.arange(sel.size) - start[bid_sorted]
        send_slot = do * BK + rank     # slot in sender's a2a_in
        recv_row = so * BK + rank      # row in receiver's a2a_out
        SBK = NCORES * BK              # multiple of 128

        # ---- send side: gather index list in slot order ----
        for c in range(NCORES):
            m = so == c
            sidx = np.zeros(SBK, np.int64)   # pad -> slot 0 (finite garbage)
            sidx[send_slot[m]] = sl[m]
            percore[c]["sendidx"][t] = _wrap16(sidx)
            percore[c]["sendidx32"][t] = np.ascontiguousarray(
                sidx.reshape(SBK // P, P).T.astype(np.int32))

        # ---- recv side: window-aligned dst-sorted tiles ----
        tiles_pw = np.zeros((NCORES, NW), np.int64)
        recv_data = []
        for c in range(NCORES):
            m = do == c
            o3 = np.argsort(dl[m], kind="stable")
            dl_s = dl[m][o3]
            row_s = recv_row[m][o3]
            w_s = w[m][o3]
            win = dl_s // P
            tiles_pw[c] = np.bincount(win, minlength=NW)
            recv_data.append((dl_s, row_s, w_s, win))
        tpw = np.maximum(1, -(-np.max(tiles_pw, axis=0) // P)).astype(np.int64)
        meta["tpw"][t] = tpw
        NT = int(tpw.sum())
        meta["NT"][t] = NT

        tile_base = np.zeros(NW + 1, np.int64)
        np.cumsum(tpw, out=tile_base[1:])
        for c in range(NCORES):
            dl_s, row_s, w_s, win = recv_data[c]
            perm = np.zeros(NT * P, np.int64)          # pad -> row 0
            smat = np.zeros((NT, P, P), np.float32)
            counts = np.zeros(NW, np.int64)
            ti = tile_base[win] + counts[win] // P     # placeholder; loop below
            counts = np.zeros(NW, np.int64)
            for k in range(dl_s.size):
                wi = win[k]
                pos = counts[wi]
                counts[wi] += 1
                tile_i = tile_base[wi] + pos // P
                mrow = pos % P
                perm[tile_i * P + mrow] = row_s[k]
                smat[tile_i, mrow, dl_s[k] % P] += w_s[k]
            percore[c]["perm"][t] = _wrap16(perm)
            percore[c]["perm32"][t] = np.ascontiguousarray(
                perm.reshape(NT, P).T.astype(np.int32))
            # smat -> [128, NT*128]: [p, j*128+col] = smat[j, p, col]
            percore[c]["smat"][t] = np.ascontiguousarray(
                np.transpose(smat, (1, 0, 2)).reshape(P, NT * P))

        # ---- touched mask ----
        for c in range(NCORES):
            deg = np.zeros(NS, np.float32)
            np.add.at(deg, sl[so == c], 1.0)
            np.add.at(deg, dl[do == c], 1.0)
            percore[c]["mask"][t] = (deg > 0).astype(np.uint8)

    return meta, percore


# ---------------------------------------------------------------------------
# device kernel builder
# ---------------------------------------------------------------------------

def _build(meta):
    STAGE = int(os.environ.get("KERNEL_STAGE", "5"))
    GMODE = os.environ.get("GMODE", "gather")
    nc = bacc.Bacc("TRN2", target_bir_lowering=False, debug=False,
                   enable_asserts=True, num_devices=NCORES)

    t_act = meta["t_active"]
    sumSB = sum(NCORES * meta["BK"][t] for t in t_act)      # send idx count
    sumNT = sum(meta["NT"][t] for t in t_act)
    BKmax = max(meta["BK"][t] for t in t_act) if t_act else 16
    NTmax = max(meta["NT"][t] for t in t_act) if t_act else 1
    SBKmax = NCORES * BKmax

    emb_in = nc.dram_tensor("embT", [2 * P, NS], f32, kind="ExternalInput").ap()
    aw_in = nc.dram_tensor("adaptW", [P, 2 * P], f32, kind="ExternalInput").ap()
    ab_in = nc.dram_tensor("adaptB", [P, 1], f32, kind="ExternalInput").ap()
    weff_in = nc.dram_tensor("weff", [P, T * L * P], bf16, kind="ExternalInput").ap()
    sidx_in = nc.dram_tensor("sendidx", [P, max(1, sumSB // 16)], i16,
                             kind="ExternalInput").ap()
    sidx32_in = nc.dram_tensor("sendidx32", [P, max(1, sumSB // P)], i32,
                               kind="ExternalInput").ap()
    perm32_in = nc.dram_tensor("perm32", [P, max(1, sumNT)], i32,
                               kind="ExternalInput").ap()
    perm_in = nc.dram_tensor("perm", [P, max(1, sumNT * P // 16)], i16,
                             kind="ExternalInput").ap()
    smat_in = nc.dram_tensor("smat", [P, max(1, sumNT) * P], bf16,
                             kind="ExternalInput").ap()
    mask_in = nc.dram_tensor("mask", [P, max(1, len(t_act)) * NS], u8,
                             kind="ExternalInput").ap()

    hT_out = nc.dram_tensor("hT_out", [P, NS], bf16, kind="ExternalOutput").ap()

    GW = max(NS, NTmax * P)

    with tile.TileContext(nc) as tc:
        with (
            tc.tile_pool(name="per", bufs=1) as per,
            tc.tile_pool(name="dram", bufs=1, space="DRAM") as dram,
        ):
            hT = per.tile([P, NS], bf16)
            hS = per.tile([P, NS], bf16)
            hwG = per.tile([P, GW], bf16)      # hw tiles, then gathered msgs
            msb = per.tile([P, SBKmax], bf16)  # staged send messages
            S = per.tile([P, NTmax * P], bf16)
            msk = per.tile([P, NS], u8)
            weff = per.tile([P, T * L * P], bf16)
            adb = per.tile([P, 1], f32)

            nc.sync.dma_start(out=weff[:], in_=weff_in[:])
            nc.sync.dma_start(out=adb[:], in_=ab_in[:])

            hw_dram = dram.tile([NS, P], bf16)
            a2a_in = dram.tile([SBKmax, P], bf16)
            a2a_out = dram.tile([SBKmax, P], bf16)

            # ---------------- h0 = embT @ adapt_W + b ----------------
            with (
                tc.tile_pool(name="h0sb", bufs=1) as h0sb,
                tc.tile_pool(name="h0ps", bufs=3, space="PSUM") as h0ps,
            ):
                awt = h0sb.tile([P, 2 * P], f32)
                nc.sync.dma_start(out=awt[:], in_=aw_in[:])
                embT = h0sb.tile([P, 2 * NS], f32)
                for ch in range(2):
                    nc.sync.dma_start(out=embT[:, ch * NS:(ch + 1) * NS],
                                      in_=emb_in[ch * P:(ch + 1) * P, :])
                for wq in range(15):
                    c0 = wq * 512
                    cw = min(512, NS - c0)
                    ps0 = h0ps.tile([P, 512], f32, space="PSUM", tag="ps0")
                    for ch in range(2):
                        nc.tensor.matmul(out=ps0[:, :cw],
                                         lhsT=awt[:, ch * P:(ch + 1) * P],
                                         rhs=embT[:, ch * NS + c0:ch * NS + c0 + cw],
                                         start=(ch == 0), stop=(ch == 1))
                    nc.vector.tensor_scalar_add(out=hT[:, c0:c0 + cw],
                                                in0=ps0[:, :cw],
                                                scalar1=adb[:, :1])

            # ---------------- T x L loop ----------------
            with (
                tc.tile_pool(name="ld", bufs=2) as ld,
                tc.tile_pool(name="tmp", bufs=4) as tmp,
                tc.tile_pool(name="pshw", bufs=4, space="PSUM") as pshw,
                tc.tile_pool(name="psagg", bufs=4, space="PSUM") as psagg,
            ):
                sb_base = 0
                nt_base = 0
                for kt, t in enumerate(t_act):
                    BK = meta["BK"][t]
                    NT = meta["NT"][t]
                    tpw = meta["tpw"][t]
                    SBK = NCORES * BK

                    nc.sync.dma_start(
                        out=S[:, :NT * P],
                        in_=smat_in[:, nt_base * P:(nt_base + NT) * P])
                    nc.sync.dma_start(
                        out=msk[:], in_=mask_in[:, kt * NS:(kt + 1) * NS])
                    sidxt = ld.tile([P, SBK // 16], i16, tag="sidxt")
                    nc.sync.dma_start(
                        out=sidxt[:],
                        in_=sidx_in[:, sb_base // 16:(sb_base + SBK) // 16])
                    permt = ld.tile([P, NT * P // 16], i16, tag="permt")
                    nc.sync.dma_start(
                        out=permt[:],
                        in_=perm_in[:, nt_base * P // 16:(nt_base + NT) * P // 16])
                    sidx32t = ld.tile([P, SBK // P], i32, tag="sidx32t")
                    nc.sync.dma_start(
                        out=sidx32t[:],
                        in_=sidx32_in[:, sb_base // P:(sb_base + SBK) // P])
                    perm32t = ld.tile([P, NT], i32, tag="perm32t")
                    nc.sync.dma_start(
                        out=perm32t[:],
                        in_=perm32_in[:, nt_base:nt_base + NT])

                    for li in range(L):
                        hsrc = hT if (li == 0 or STAGE < 5) else hS
                        Wt = weff[:, (t * L + li) * P:(t * L + li + 1) * P]

                        # 1) hw = h @ W_eff (node-major tiles) -> SBUF -> DRAM
                        for wq in range(NW):
                            ps = pshw.tile([P, P], f32, space="PSUM", tag="pshw")
                            nc.tensor.matmul(out=ps[:],
                                             lhsT=hsrc[:, wq * P:(wq + 1) * P],
                                             rhs=Wt, start=True, stop=True)
                            nc.vector.tensor_copy(
                                out=hwG[:, wq * P:(wq + 1) * P], in_=ps[:])
                        nc.sync.dma_start(
                            out=hw_dram[:].rearrange("(w q) f -> q w f", q=P),
                            in_=hwG[:, :NS].rearrange("p (w f) -> p w f", f=P))

                        # 2) send-side gather into slot order + stage to a2a_in
                        if STAGE < 2:
                            continue
                        if GMODE == "indirect":
                            for b in range(SBK // P):
                                nc.gpsimd.indirect_dma_start(
                                    out=msb[:, b * P:(b + 1) * P],
                                    out_offset=None,
                                    in_=hw_dram[:],
                                    in_offset=bass.IndirectOffsetOnAxis(
                                        ap=sidx32t[:, b:b + 1], axis=0),
                                )
                        else:
                          for g0 in range(0, SBK, GCH):
                            gn = min(GCH, SBK - g0)
                            nc.gpsimd.dma_gather(
                                out_ap=msb[:, g0:g0 + gn].rearrange(
                                    "p (j f) -> p j f", f=P),
                                in_ap=hw_dram[:],
                                idxs_ap=sidxt[:, g0 // 16:(g0 + gn) // 16],
                                num_idxs=gn,
                                num_idxs_reg=gn,
                                elem_size=P,
                            )
                        del g0
                        if STAGE < 3:
                            continue
                        nc.sync.dma_start(
                            out=a2a_in[0:SBK, :].rearrange("(j q) f -> q j f", q=P),
                            in_=msb[:, :SBK].rearrange("p (j f) -> p j f", f=P))

                        # 3) AllToAll
                        nc.gpsimd.collective_compute(
                            "AllToAll", mybir.AluOpType.bypass,
                            replica_groups=[list(range(NCORES))],
                            ins=[a2a_in[0:SBK, :]],
                            outs=[a2a_out[0:SBK, :]],
                        )

                        # 4) recv-side gather (dst-sorted window-aligned)
                        if STAGE < 4:
                            continue
                        if GMODE == "indirect":
                            for j in range(NT):
                                nc.gpsimd.indirect_dma_start(
                                    out=hwG[:, j * P:(j + 1) * P],
                                    out_offset=None,
                                    in_=a2a_out[0:SBK, :],
                                    in_offset=bass.IndirectOffsetOnAxis(
                                        ap=perm32t[:, j:j + 1], axis=0),
                                )
                        else:
                          for g0 in range(0, NT * P, GCH):
                            gn = min(GCH, NT * P - g0)
                            nc.gpsimd.dma_gather(
                                out_ap=hwG[:, g0:g0 + gn].rearrange(
                                    "p (j f) -> p j f", f=P),
                                in_ap=a2a_out[0:SBK, :],
                                idxs_ap=permt[:, g0 // 16:(g0 + gn) // 16],
                                num_idxs=gn,
                                num_idxs_reg=gn,
                                elem_size=P,
                            )

                        # 5) S-matmuls + relu (+ mask merge on last layer)
                        if STAGE < 5:
                            continue
                        jj = 0
                        for wq in range(NW):
                            k = int(tpw[wq])
                            ps = psagg.tile([P, P], f32, space="PSUM", tag="psagg")
                            for u in range(k):
                                nc.tensor.matmul(
                                    out=ps[:],
                                    lhsT=hwG[:, (jj + u) * P:(jj + u + 1) * P],
                                    rhs=S[:, (jj + u) * P:(jj + u + 1) * P],
                                    start=(u == 0), stop=(u == k - 1))
                            jj += k
                            sl_ = slice(wq * P, (wq + 1) * P)
                            if li < L - 1:
                                nc.scalar.activation(
                                    hS[:, sl_], ps[:],
                                    mybir.ActivationFunctionType.Relu)
                            else:
                                rt = tmp.tile([P, P], f32, tag="rt")
                                nc.scalar.activation(
                                    rt[:], ps[:],
                                    mybir.ActivationFunctionType.Relu)
                                nc.vector.select(hT[:, sl_], msk[:, sl_],
                                                 rt[:], hT[:, sl_])

                    sb_base += SBK
                    nt_base += NT

                nc.sync.dma_start(out=hT_out[:], in_=hT[:])

    nc.finalize()
    return nc


# ---------------------------------------------------------------------------
# top-level kernel
# ---------------------------------------------------------------------------

def kernel(word_ids, src, dst, edge_time, edge_weight, graph_id, y_data,
           word_embeds, adapt_W, adapt_b, gcn_weights,
           lstm_Wih, lstm_Whh, lstm_bih, lstm_bhh, out_W, out_b):
    global LAST_EXEC_NS, LAST_NC

    word_ids = np.asarray(word_ids).astype(np.int64)
    src = np.asarray(src).astype(np.int64)
    dst = np.asarray(dst).astype(np.int64)
    edge_time = np.asarray(edge_time).astype(np.int64)
    edge_weight = np.asarray(edge_weight).astype(np.float32)
    graph_id = np.asarray(graph_id).astype(np.int64)
    y_data = np.asarray(y_data).astype(np.float32)
    word_embeds = np.asarray(word_embeds).astype(np.float32)
    adapt_W = np.asarray(adapt_W).astype(np.float32)
    adapt_b = np.asarray(adapt_b).astype(np.float32)
    gcn_weights = np.asarray(gcn_weights).astype(np.float32)
    lstm_Wih = np.asarray(lstm_Wih).astype(np.float32)
    lstm_Whh = np.asarray(lstm_Whh).astype(np.float32)
    lstm_bih = np.asarray(lstm_bih).astype(np.float32)
    lstm_bhh = np.asarray(lstm_bhh).astype(np.float32)
    out_W = np.asarray(out_W).astype(np.float32)
    out_b = np.asarray(out_b).astype(np.float32)

    W_eff = _evolve_weights(edge_time, gcn_weights, lstm_Wih, lstm_Whh,
                            lstm_bih, lstm_bhh)
    meta, percore = _preprocess_edges(src, dst, edge_time, edge_weight)
    trunc = os.environ.get("KERNEL_TRUNC")
    if trunc is not None:
        meta["t_active"] = meta["t_active"][:int(trunc)]
    t_act = meta["t_active"]

    weff_sw = np.zeros((P, T * L * P), ml_dtypes.bfloat16)
    for t in range(T):
        for i in range(L):
            weff_sw[:, (t * L + i) * P:(t * L + i + 1) * P] = W_eff[t][i]
    aw_sw = np.ascontiguousarray(
        adapt_W.reshape(2, P, P).transpose(1, 0, 2).reshape(P, 2 * P))
    ab = np.ascontiguousarray(adapt_b.reshape(P, 1))

    sumSB = sum(NCORES * meta["BK"][t] for t in t_act)
    sumNT = sum(meta["NT"][t] for t in t_act)
    nmask = max(1, len(t_act))

    in_maps = []
    for c in range(NCORES):
        wid = word_ids[c * NPC:(c + 1) * NPC]
        embT = np.zeros((2 * P, NS), np.float32)
        ge = word_embeds[wid]
        embT[0:P, 0:NPC] = ge[:, 0:P].T
        embT[P:2 * P, 0:NPC] = ge[:, P:2 * P].T

        sidx = np.zeros((128, max(1, sumSB // 16)), np.int16)
        sidx32 = np.zeros((128, max(1, sumSB // P)), np.int32)
        perm32c = np.zeros((128, max(1, sumNT)), np.int32)
        permc = np.zeros((128, max(1, sumNT * P // 16)), np.int16)
        smatc = np.zeros((P, max(1, sumNT) * P), ml_dtypes.bfloat16)
        maskc = np.zeros((P, nmask * NS), np.uint8)
        sb = 0
        ntb = 0
        for kt, t in enumerate(t_act):
            SBK = NCORES * meta["BK"][t]
            NT = meta["NT"][t]
            sidx[:, sb // 16:(sb + SBK) // 16] = percore[c]["sendidx"][t]
            sidx32[:, sb // P:(sb + SBK) // P] = percore[c]["sendidx32"][t]
            perm32c[:, ntb:ntb + NT] = percore[c]["perm32"][t]
            permc[:, ntb * P // 16:(ntb + NT) * P // 16] = percore[c]["perm"][t]
            smatc[:, ntb * P:(ntb + NT) * P] = percore[c]["smat"][t]
            maskc[:, kt * NS:(kt + 1) * NS] = percore[c]["mask"][t][None, :]
            sb += SBK
            ntb += NT

        in_maps.append({
            "embT": embT, "adaptW": aw_sw, "adaptB": ab, "weff": weff_sw,
            "sendidx": sidx, "sendidx32": sidx32, "perm": permc,
            "perm32": perm32c, "smat": smatc, "mask": maskc,
        })

    nc = _build(meta)
    LAST_NC = nc
    prof_ctx = contextlib.nullcontext()
    prof_dir = os.environ.get("KERNEL_PROF_DIR")
    if prof_dir:
        try:
            from trn_agent_boot.trn_boot import _ntff_profile_via_ctypes
            hook = _ntff_profile_via_ctypes("/opt/axon/libaxon_pjrt.so")
            if hook is not None:
                os.makedirs(prof_dir, exist_ok=True)
                prof_ctx = hook(prof_dir, None)
        except Exception as e:  # profiling is best-effort only
            print(f"profiling hook unavailable: {e}")
    with prof_ctx:
        res = run_bass_kernel_spmd(nc, in_maps, core_ids=list(range(NCORES)))
    LAST_EXEC_NS = res.exec_time_ns

    h = np.zeros((N, D), np.float32)
    for c in range(NCORES):
        h[c * NPC:(c + 1) * NPC] = res.results[c]["hT_out"][:, :NPC].T.astype(np.float32)

    pooled = np.full((B, D), -np.inf, np.float32)
    np.maximum.at(pooled, graph_id, h)
    pooled = np.where(np.isfinite(pooled), pooled, 0.0).astype(np.float32)
    logits = (pooled @ out_W + out_b).reshape(-1).astype(np.float32)
    loss = np.mean(np.maximum(logits, 0.0) - logits * y_data +
                   np.log1p(np.exp(-np.abs(logits)))).astype(np.float32)
    probs = _sigmoid(logits).astype(np.float32)
    return loss, probs


# revision 25
# speedup vs baseline: 1.3967x; 1.3482x over previous
"""EvolveGCN forward on 8 Trainium2 NeuronCores (Bass/Tile).

Strategy (graph/data parallel, per the sharding hint):
 - Nodes sharded contiguously across 8 cores (7500 real + 52 pad slots each).
 - Node features h kept FEATURE-MAJOR in SBUF: hT [128 feat x 7552 nodes].
 - Per (t, layer): hw = h @ W_eff computed as 59 stationary-load matmuls
   (lhsT = hT window slice) giving NODE-major hw tiles -> SBUF -> DRAM.
 - Send side: dma_gather of hw rows per out-edge in AllToAll-slot order
   (bucketed by destination owner), staged to the A2A input buffer.
 - AllToAll collective exchanges message rows between the 8 cores.
 - Receive side: dma_gather of received rows in dst-sorted 128-node-window-
   aligned order; per window one matmul aggT_w = msgs^T @ S_w (S carries the
   edge weights, built on host) performs scatter-sum + weighting + transpose
   back to feature-major in a single PE op.
 - relu on ACT engine PSUM->SBUF; end-of-timestep touched-mask select on DVE.
 - Evolved weights (LSTM chain over the 128x128 GCN weights) are a pure
   function of the parameters; computed on host, replicated to all cores.
 - Final graph max-pool + output head + BCE computed on host from the
   returned per-core hT (the gather/unshard step).
"""
import contextlib
import os

import ml_dtypes
import numpy as np

import concourse.bass as bass
import concourse.bacc as bacc
import concourse.mybir as mybir
import concourse.tile as tile
from concourse.bass_utils import run_bass_kernel_spmd

# problem constants (hardcoded per spec)
N, E, D, NINP, V, T, L, B = 60000, 200000, 128, 256, 15000, 6, 2, 16
NCORES = 8
NPC = N // NCORES          # 7500 real nodes per core
P = 128
NW = 59                    # windows per core (59*128 = 7552 slots)
NS = NW * P                # 7552 padded node slots per core
GCH = 1024                 # dma_gather chunk (indices per call)

f32 = mybir.dt.float32
u8 = mybir.dt.uint8
i32 = mybir.dt.int32
i16 = mybir.dt.int16
bf16 = mybir.dt.bfloat16

LAST_EXEC_NS = None
LAST_NC = None


# ---------------------------------------------------------------------------
# host-side math (exact replica of reference semantics, numpy fp32)
# ---------------------------------------------------------------------------

def _sigmoid(x):
    return 1.0 / (1.0 + np.exp(-x))


def _lstm_cell(x, h, c, Wih, Whh, bih, bhh):
    g = x @ Wih.T + bih + h @ Whh.T + bhh
    i, f, gg, o = np.split(g, 4, axis=-1)
    c2 = _sigmoid(f) * c + _sigmoid(i) * np.tanh(gg)
    return _sigmoid(o) * np.tanh(c2), c2


def _evolve_weights(edge_time, gcn_weights, lstm_Wih, lstm_Whh, lstm_bih, lstm_bhh):
    """W_eff[t][i]: the evolved GCN weight used at step t, layer i."""
    hx = [gcn_weights[i].copy() for i in range(L)]
    cx = [np.zeros((D, D), np.float32) for _ in range(L)]
    z = np.zeros((D, D), np.float32)
    first = False
    W_eff = []
    for t in range(T):
        has = bool(np.any(edge_time == t))
        row = []
        for i in range(L):
            h_zero, c_zero = _lstm_cell(hx[i], z, z, lstm_Wih[i], lstm_Whh[i],
                                        lstm_bih[i], lstm_bhh[i])
            h_st, _ = _lstm_cell(hx[i], hx[i], cx[i], lstm_Wih[i], lstm_Whh[i],
                                 lstm_bih[i], lstm_bhh[i])
            hi = h_st if first else h_zero
            ci = cx[i] if first else c_zero
            if not has:
                hi = hx[i]
                ci = cx[i]
            hx[i] = np.asarray(hi, np.float32)
            cx[i] = np.asarray(ci, np.float32)
            row.append(hx[i])
        W_eff.append(row)
        first = first or has
    return W_eff


def _wrap16(idx):
    """Pack an index list into the [128, n/16] wrapped SBUF layout used by
    dma_gather: index i lands at [i % 16, i // 16]; rows 16..127 zero."""
    n = idx.size
    assert n % 16 == 0
    return np.ascontiguousarray(
        np.tile(idx.reshape(n // 16, 16).T.astype(np.int16), (8, 1)))


# ---------------------------------------------------------------------------
# host-side graph preprocessing
# ---------------------------------------------------------------------------

def _preprocess_edges(src, dst, edge_time, edge_weight):
    """Static per-t structures; sizes uniform across cores."""
    meta = {"t_active": [], "BK": {}, "NT": {}, "NP": {}, "tpw": {}}
    percore = [dict(sendidx={}, sendidx32={}, perm={}, perm32={}, smat={}, mask={}) for _ in range(NCORES)]

    for t in range(T):
        sel = np.where(edge_time == t)[0]
        if sel.size == 0:
            continue
        meta["t_active"].append(t)
        s = src[sel].astype(np.int64)
        d = dst[sel].astype(np.int64)
        w = edge_weight[sel].astype(np.float32)
        so = s // NPC
        do = d // NPC
        sl = s % NPC
        dl = d % NPC

        bucket_id = so * NCORES + do
        order = np.argsort(bucket_id, kind="stable")
        bid_sorted = bucket_id[order]
        cnt = np.bincount(bid_sorted, minlength=NCORES * NCORES)
        BK = int(((cnt.max() + 15) // 16) * 16)
        meta["BK"][t] = BK
        start = np.zeros(NCORES * NCORES + 1, np.int64)
        np.cumsum(cnt, out=start[1:])
        rank = np.empty(sel.size, np.int64)
        rank[order] = np.arange(sel.size) - start[bid_sorted]
        send_slot = do * BK + rank     # slot in sender's a2a_in
        recv_row = so * BK + rank      # row in receiver's a2a_out
        SBK = NCORES * BK              # multiple of 128

        # ---- send side: gather index list in slot order ----
        for c in range(NCORES):
            m = so == c
            sidx = np.zeros(SBK, np.int64)   # pad -> slot 0 (finite garbage)
            sidx[send_slot[m]] = sl[m]
            percore[c]["sendidx"][t] = _wrap16(sidx)


        # ---- recv side: window-aligned dst-sorted tiles ----
        tiles_pw = np.zeros((NCORES, NW), np.int64)
        recv_data = []
        for c in range(NCORES):
            m = do == c
            o3 = np.argsort(dl[m], kind="stable")
            dl_s = dl[m][o3]
            row_s = recv_row[m][o3]
            w_s = w[m][o3]
            win = dl_s // P
            tiles_pw[c] = np.bincount(win, minlength=NW)
            recv_data.append((dl_s, row_s, w_s, win))
        mw = np.max(tiles_pw, axis=0).astype(np.int64)   # per-window uniform count
        off = np.zeros(NW + 1, np.int64)
        np.cumsum(mw, out=off[1:])
        MS = int(off[-1])
        NTC = max(1, -(-MS // P))                         # compact gather tiles
        meta["NT"][t] = NTC
        # uniform (window -> spanning tiles) pair structure
        pair_tiles = []
        for wq in range(NW):
            lo, hi = int(off[wq]), int(off[wq] + mw[wq])
            js = list(range(lo // P, -(-hi // P))) if hi > lo else [0]
            pair_tiles.append(js)
        meta["tpw"][t] = pair_tiles
        NP = sum(len(js) for js in pair_tiles)
        meta["NP"][t] = NP
        pb = np.zeros(NW + 1, np.int64)
        np.cumsum([len(js) for js in pair_tiles], out=pb[1:])
        for c in range(NCORES):
            dl_s, row_s, w_s, win = recv_data[c]
            perm = np.zeros(NTC * P, np.int64)            # pad -> row 0
            smat = np.zeros((NP, P, P), np.float32)
            counts = np.zeros(NW, np.int64)
            for k in range(dl_s.size):
                wi = win[k]
                slot = off[wi] + counts[wi]
                counts[wi] += 1
                perm[slot] = row_s[k]
                j = slot // P
                pidx = pb[wi] + (j - pair_tiles[wi][0])
                smat[pidx, slot % P, dl_s[k] % P] += w_s[k]
            percore[c]["perm"][t] = _wrap16(perm)
            percore[c]["smat"][t] = np.ascontiguousarray(
                np.transpose(smat, (1, 0, 2)).reshape(P, NP * P))

        # ---- touched mask ----
        for c in range(NCORES):
            deg = np.zeros(NS, np.float32)
            np.add.at(deg, sl[so == c], 1.0)
            np.add.at(deg, dl[do == c], 1.0)
            percore[c]["mask"][t] = (deg > 0).astype(np.uint8)

    return meta, percore


# ---------------------------------------------------------------------------
# device kernel builder
# ---------------------------------------------------------------------------

def _build(meta):
    STAGE = int(os.environ.get("KERNEL_STAGE", "5"))
    GMODE = os.environ.get("GMODE", "gather")
    nc = bacc.Bacc("TRN2", target_bir_lowering=False, debug=False,
                   enable_asserts=True, num_devices=NCORES)

    t_act = meta["t_active"]
    sumSB = sum(NCORES * meta["BK"][t] for t in t_act)      # send idx count
    sumNT = sum(meta["NT"][t] for t in t_act)
    sumNP = sum(meta["NP"][t] for t in t_act)
    BKmax = max(meta["BK"][t] for t in t_act) if t_act else 16
    NTmax = max(meta["NT"][t] for t in t_act) if t_act else 1
    NPmax = max(meta["NP"][t] for t in t_act) if t_act else 1
    SBKmax = NCORES * BKmax

    emb_in = nc.dram_tensor("embT", [2 * P, NS], f32, kind="ExternalInput").ap()
    aw_in = nc.dram_tensor("adaptW", [P, 2 * P], f32, kind="ExternalInput").ap()
    ab_in = nc.dram_tensor("adaptB", [P, 1], f32, kind="ExternalInput").ap()
    weff_in = nc.dram_tensor("weff", [P, T * L * P], bf16, kind="ExternalInput").ap()
    sidx_in = nc.dram_tensor("sendidx", [P, max(1, sumSB // 16)], i16,
                             kind="ExternalInput").ap()

    perm_in = nc.dram_tensor("perm", [P, max(1, sumNT * P // 16)], i16,
                             kind="ExternalInput").ap()
    smat_in = nc.dram_tensor("smat", [P, max(1, sumNP) * P], bf16,
                             kind="ExternalInput").ap()
    mask_in = nc.dram_tensor("mask", [P, max(1, len(t_act)) * NS], u8,
                             kind="ExternalInput").ap()

    hT_out = nc.dram_tensor("hT_out", [P, NS], bf16, kind="ExternalOutput").ap()

    GW = max(NS, NTmax * P)

    with tile.TileContext(nc) as tc:
        with (
            tc.tile_pool(name="per", bufs=1) as per,
            tc.tile_pool(name="dram", bufs=1, space="DRAM") as dram,
        ):
            hT = per.tile([P, NS], bf16)
            hS = per.tile([P, NS], bf16)
            hwG = per.tile([P, GW], bf16)      # hw tiles, then gathered msgs
            msb = per.tile([P, SBKmax], bf16)  # staged send messages
            S = per.tile([P, NPmax * P], bf16)
            msk = per.tile([P, NS], u8)
            weff = per.tile([P, T * L * P], bf16)
            adb = per.tile([P, 1], f32)

            nc.sync.dma_start(out=weff[:], in_=weff_in[:])
            nc.sync.dma_start(out=adb[:], in_=ab_in[:])

            hw_dram = dram.tile([NS, P], bf16)
            a2a_in = dram.tile([SBKmax, P], bf16)
            a2a_out = dram.tile([SBKmax, P], bf16)

            # ---------------- h0 = embT @ adapt_W + b ----------------
            with (
                tc.tile_pool(name="h0sb", bufs=1) as h0sb,
                tc.tile_pool(name="h0ps", bufs=3, space="PSUM") as h0ps,
            ):
                awt = h0sb.tile([P, 2 * P], f32)
                nc.sync.dma_start(out=awt[:], in_=aw_in[:])
                embT = h0sb.tile([P, 2 * NS], f32)
                for ch in range(2):
                    nc.sync.dma_start(out=embT[:, ch * NS:(ch + 1) * NS],
                                      in_=emb_in[ch * P:(ch + 1) * P, :])
                for wq in range(15):
                    c0 = wq * 512
                    cw = min(512, NS - c0)
                    ps0 = h0ps.tile([P, 512], f32, space="PSUM", tag="ps0")
                    for ch in range(2):
                        nc.tensor.matmul(out=ps0[:, :cw],
                                         lhsT=awt[:, ch * P:(ch + 1) * P],
                                         rhs=embT[:, ch * NS + c0:ch * NS + c0 + cw],
                                         start=(ch == 0), stop=(ch == 1))
                    nc.vector.tensor_scalar_add(out=hT[:, c0:c0 + cw],
                                                in0=ps0[:, :cw],
                                                scalar1=adb[:, :1])

            # ---------------- T x L loop ----------------
            with (
                tc.tile_pool(name="ld", bufs=2) as ld,
                tc.tile_pool(name="tmp", bufs=4) as tmp,
                tc.tile_pool(name="pshw", bufs=4, space="PSUM") as pshw,
                tc.tile_pool(name="psagg", bufs=4, space="PSUM") as psagg,
            ):
                sb_base = 0
                nt_base = 0
                np_base = 0
                for kt, t in enumerate(t_act):
                    BK = meta["BK"][t]
                    NT = meta["NT"][t]
                    NP_ = meta["NP"][t]
                    pair_tiles = meta["tpw"][t]
                    SBK = NCORES * BK

                    nc.sync.dma_start(
                        out=S[:, :NP_ * P],
                        in_=smat_in[:, np_base * P:(np_base + NP_) * P])
                    nc.sync.dma_start(
                        out=msk[:], in_=mask_in[:, kt * NS:(kt + 1) * NS])
                    sidxt = ld.tile([P, SBK // 16], i16, tag="sidxt")
                    nc.sync.dma_start(
                        out=sidxt[:],
                        in_=sidx_in[:, sb_base // 16:(sb_base + SBK) // 16])
                    permt = ld.tile([P, NT * P // 16], i16, tag="permt")
                    nc.sync.dma_start(
                        out=permt[:],
                        in_=perm_in[:, nt_base * P // 16:(nt_base + NT) * P // 16])


                    for li in range(L):
                        hsrc = hT if (li == 0 or STAGE < 5) else hS
                        Wt = weff[:, (t * L + li) * P:(t * L + li + 1) * P]

                        # 1) hw = h @ W_eff (node-major tiles) -> SBUF -> DRAM
                        for wq in range(NW):
                            ps = pshw.tile([P, P], f32, space="PSUM", tag="pshw")
                            nc.tensor.matmul(out=ps[:],
                                             lhsT=hsrc[:, wq * P:(wq + 1) * P],
                                             rhs=Wt, start=True, stop=True)
                            nc.vector.tensor_copy(
                                out=hwG[:, wq * P:(wq + 1) * P], in_=ps[:])
                        nc.sync.dma_start(
                            out=hw_dram[:].rearrange("(w q) f -> q w f", q=P),
                            in_=hwG[:, :NS].rearrange("p (w f) -> p w f", f=P))

                        # 2) send-side gather into slot order + stage to a2a_in
                        if STAGE < 2:
                            continue
                        if GMODE == "indirect":
                            for b in range(SBK // P):
                                nc.gpsimd.indirect_dma_start(
                                    out=msb[:, b * P:(b + 1) * P],
                                    out_offset=None,
                                    in_=hw_dram[:],
                                    in_offset=bass.IndirectOffsetOnAxis(
                                        ap=sidx32t[:, b:b + 1], axis=0),
                                )
                        else:
                          for g0 in range(0, SBK, GCH):
                            gn = min(GCH, SBK - g0)
                            nc.gpsimd.dma_gather(
                                out_ap=msb[:, g0:g0 + gn].rearrange(
                                    "p (j f) -> p j f", f=P),
                                in_ap=hw_dram[:],
                                idxs_ap=sidxt[:, g0 // 16:(g0 + gn) // 16],
                                num_idxs=gn,
                                num_idxs_reg=gn,
                                elem_size=P,
                            )
                        del g0
                        if STAGE < 3:
                            continue
                        nc.sync.dma_start(
                            out=a2a_in[0:SBK, :].rearrange("(j q) f -> q j f", q=P),
                            in_=msb[:, :SBK].rearrange("p (j f) -> p j f", f=P))

                        # 3) AllToAll
                        nc.gpsimd.collective_compute(
                            "AllToAll", mybir.AluOpType.bypass,
                            replica_groups=[list(range(NCORES))],
                            ins=[a2a_in[0:SBK, :]],
                            outs=[a2a_out[0:SBK, :]],
                        )

                        # 4) recv-side gather (dst-sorted window-aligned)
                        if STAGE < 4:
                            continue
                        if GMODE == "indirect":
                            for j in range(NT):
                                nc.gpsimd.indirect_dma_start(
                                    out=hwG[:, j * P:(j + 1) * P],
                                    out_offset=None,
                                    in_=a2a_out[0:SBK, :],
                                    in_offset=bass.IndirectOffsetOnAxis(
                                        ap=perm32t[:, j:j + 1], axis=0),
                                )
                        else:
                          for g0 in range(0, NT * P, GCH):
                            gn = min(GCH, NT * P - g0)
                            nc.gpsimd.dma_gather(
                                out_ap=hwG[:, g0:g0 + gn].rearrange(
                                    "p (j f) -> p j f", f=P),
                                in_ap=a2a_out[0:SBK, :],
                                idxs_ap=permt[:, g0 // 16:(g0 + gn) // 16],
                                num_idxs=gn,
                                num_idxs_reg=gn,
                                elem_size=P,
                            )

                        # 5) S-matmuls + relu (+ mask merge on last layer)
                        if STAGE < 5:
                            continue
                        jj = 0
                        for wq in range(NW):
                            js = pair_tiles[wq]
                            ps = psagg.tile([P, P], f32, space="PSUM", tag="psagg")
                            for u, jt in enumerate(js):
                                nc.tensor.matmul(
                                    out=ps[:],
                                    lhsT=hwG[:, jt * P:(jt + 1) * P],
                                    rhs=S[:, (jj + u) * P:(jj + u + 1) * P],
                                    start=(u == 0), stop=(u == len(js) - 1))
                            jj += len(js)
                            sl_ = slice(wq * P, (wq + 1) * P)
                            if li < L - 1:
                                nc.scalar.activation(
                                    hS[:, sl_], ps[:],
                                    mybir.ActivationFunctionType.Relu)
                            else:
                                rt = tmp.tile([P, P], f32, tag="rt")
                                nc.scalar.activation(
                                    rt[:], ps[:],
                                    mybir.ActivationFunctionType.Relu)
                                nc.vector.select(hT[:, sl_], msk[:, sl_],
                                                 rt[:], hT[:, sl_])

                    sb_base += SBK
                    nt_base += NT
                    np_base += NP_

                nc.sync.dma_start(out=hT_out[:], in_=hT[:])

    nc.finalize()
    return nc


# ---------------------------------------------------------------------------
# top-level kernel
# ---------------------------------------------------------------------------

def kernel(word_ids, src, dst, edge_time, edge_weight, graph_id, y_data,
           word_embeds, adapt_W, adapt_b, gcn_weights,
           lstm_Wih, lstm_Whh, lstm_bih, lstm_bhh, out_W, out_b):
    global LAST_EXEC_NS, LAST_NC

    word_ids = np.asarray(word_ids).astype(np.int64)
    src = np.asarray(src).astype(np.int64)
    dst = np.asarray(dst).astype(np.int64)
    edge_time = np.asarray(edge_time).astype(np.int64)
    edge_weight = np.asarray(edge_weight).astype(np.float32)
    graph_id = np.asarray(graph_id).astype(np.int64)
    y_data = np.asarray(y_data).astype(np.float32)
    word_embeds = np.asarray(word_embeds).astype(np.float32)
    adapt_W = np.asarray(adapt_W).astype(np.float32)
    adapt_b = np.asarray(adapt_b).astype(np.float32)
    gcn_weights = np.asarray(gcn_weights).astype(np.float32)
    lstm_Wih = np.asarray(lstm_Wih).astype(np.float32)
    lstm_Whh = np.asarray(lstm_Whh).astype(np.float32)
    lstm_bih = np.asarray(lstm_bih).astype(np.float32)
    lstm_bhh = np.asarray(lstm_bhh).astype(np.float32)
    out_W = np.asarray(out_W).astype(np.float32)
    out_b = np.asarray(out_b).astype(np.float32)

    W_eff = _evolve_weights(edge_time, gcn_weights, lstm_Wih, lstm_Whh,
                            lstm_bih, lstm_bhh)
    meta, percore = _preprocess_edges(src, dst, edge_time, edge_weight)
    trunc = os.environ.get("KERNEL_TRUNC")
    if trunc is not None:
        meta["t_active"] = meta["t_active"][:int(trunc)]
    t_act = meta["t_active"]

    weff_sw = np.zeros((P, T * L * P), ml_dtypes.bfloat16)
    for t in range(T):
        for i in range(L):
            weff_sw[:, (t * L + i) * P:(t * L + i + 1) * P] = W_eff[t][i]
    aw_sw = np.ascontiguousarray(
        adapt_W.reshape(2, P, P).transpose(1, 0, 2).reshape(P, 2 * P))
    ab = np.ascontiguousarray(adapt_b.reshape(P, 1))

    sumSB = sum(NCORES * meta["BK"][t] for t in t_act)
    sumNT = sum(meta["NT"][t] for t in t_act)
    sumNP = sum(meta["NP"][t] for t in t_act)
    nmask = max(1, len(t_act))

    in_maps = []
    for c in range(NCORES):
        wid = word_ids[c * NPC:(c + 1) * NPC]
        embT = np.zeros((2 * P, NS), np.float32)
        ge = word_embeds[wid]
        embT[0:P, 0:NPC] = ge[:, 0:P].T
        embT[P:2 * P, 0:NPC] = ge[:, P:2 * P].T

        sidx = np.zeros((128, max(1, sumSB // 16)), np.int16)

        permc = np.zeros((128, max(1, sumNT * P // 16)), np.int16)
        smatc = np.zeros((P, max(1, sumNP) * P), ml_dtypes.bfloat16)
        maskc = np.zeros((P, nmask * NS), np.uint8)
        sb = 0
        ntb = 0
        npb = 0
        for kt, t in enumerate(t_act):
            SBK = NCORES * meta["BK"][t]
            NT = meta["NT"][t]
            sidx[:, sb // 16:(sb + SBK) // 16] = percore[c]["sendidx"][t]
            NP_ = meta["NP"][t]
            permc[:, ntb * P // 16:(ntb + NT) * P // 16] = percore[c]["perm"][t]
            smatc[:, npb * P:(npb + NP_) * P] = percore[c]["smat"][t]
            maskc[:, kt * NS:(kt + 1) * NS] = percore[c]["mask"][t][None, :]
            sb += SBK
            ntb += NT
            npb += NP_

        in_maps.append({
            "embT": embT, "adaptW": aw_sw, "adaptB": ab, "weff": weff_sw,
            "sendidx": sidx, "perm": permc, "smat": smatc, "mask": maskc,
        })

    nc = _build(meta)
    LAST_NC = nc
    prof_ctx = contextlib.nullcontext()
    prof_dir = os.environ.get("KERNEL_PROF_DIR")
    if prof_dir:
        try:
            from trn_agent_boot.trn_boot import _ntff_profile_via_ctypes
            hook = _ntff_profile_via_ctypes("/opt/axon/libaxon_pjrt.so")
            if hook is not None:
                os.makedirs(prof_dir, exist_ok=True)
                prof_ctx = hook(prof_dir, None)
        except Exception as e:  # profiling is best-effort only
            print(f"profiling hook unavailable: {e}")
    with prof_ctx:
        res = run_bass_kernel_spmd(nc, in_maps, core_ids=list(range(NCORES)))
    LAST_EXEC_NS = res.exec_time_ns

    h = np.zeros((N, D), np.float32)
    for c in range(NCORES):
        h[c * NPC:(c + 1) * NPC] = res.results[c]["hT_out"][:, :NPC].T.astype(np.float32)

    pooled = np.full((B, D), -np.inf, np.float32)
    np.maximum.at(pooled, graph_id, h)
    pooled = np.where(np.isfinite(pooled), pooled, 0.0).astype(np.float32)
    logits = (pooled @ out_W + out_b).reshape(-1).astype(np.float32)
    loss = np.mean(np.maximum(logits, 0.0) - logits * y_data +
                   np.log1p(np.exp(-np.abs(logits)))).astype(np.float32)
    probs = _sigmoid(logits).astype(np.float32)
    return loss, probs
